# revision 12
# baseline (speedup 1.0000x reference)
"""Trainium2 Bass kernel for the DifferentiableDAG dense-MLP problem.

Computation (per variable i of V=512):
    adj = (sigmoid(logits) > 0.5) * (1 - eye)        # == (logits > 0) * (1-eye)
    W1_eff[i,v,h] = W1[i,v,h] * adj[v,i]
    h[b,i,:]   = relu(X @ W1_eff[i] + b1[i])         # [B,H]
    out[b,i]   = h[b,i,:] @ W2[i] + b2[i]            # [B]
returns (out [B,V], adj [V,V]).

Sharding: expert-parallel over the variable axis i — core c owns the 64
variables [64c, 64c+64).  X^T is replicated; outputs are gathered on host.

Per-core kernel layout ("transposed h"):
  layer 1: out_psum[(pair_var, h), b] = sum_v W1_eff^T X^T — stationary is the
  masked W1 pair tile [128v, 128=(2 vars x 64 h)], moving is X^T [128v, 512b],
  accumulated over 4 v-chunks.  Bias+relu is then ONE fused DVE tensor_scalar
  op (bias is per-partition in this layout).
  layer 2: block-diagonal W2 stationary [128=(2x64h), 32] per PSUM col-group
  strip; 8 pairs accumulate into each 32-partition strip of one PSUM bank.
"""

import numpy as np

import concourse.bass as bass
import concourse.mybir as mybir
import concourse.tile as tile
from contextlib import ExitStack

B, V, H = 4096, 512, 64
NCORES = 8
VL = V // NCORES          # 64 local variables per core
PAIRS = VL // 2           # 32
GROUPS = 4                # PSUM col-group strips for layer 2
PERG = PAIRS // GROUPS    # 8 pairs accumulated per strip
KC = V // 128             # 4 contraction chunks
BTS = 512                 # batch tile size
NBT = B // BTS            # 8 batch tiles

f32 = mybir.dt.float32

# Matmul input dtype: float32r streams at 1 cycle/row for N>=256 (4x faster
# than plain float32 on TRN2); numerics verified against the fp32 reference.
MM_DTYPE = mybir.dt.float32r




def build_program(repeat=1):
    """repeat>1 re-emits the main batch loop for slope-based HW timing."""
    from concourse import bacc
    nc = bacc.Bacc(
        "TRN2", target_bir_lowering=False, debug=False, num_devices=NCORES
    )

    XT = nc.declare_dram_parameter("XT", [V, B], f32, isOutput=False)
    W1s = nc.declare_dram_parameter("W1s", [VL, V, H], f32, isOutput=False)
    b1s = nc.declare_dram_parameter("b1s", [VL, H], f32, isOutput=False)
    W2s = nc.declare_dram_parameter("W2s", [VL, H], f32, isOutput=False)
    b2s = nc.declare_dram_parameter("b2s", [VL], f32, isOutput=False)
    LG = nc.declare_dram_parameter("LG", [V, VL], f32, isOutput=False)
    DM = nc.declare_dram_parameter("DM", [V, VL], f32, isOutput=False)
    outT = nc.declare_dram_parameter("outT", [VL, B], f32, isOutput=True)
    adjO = nc.declare_dram_parameter("adjO", [V, VL], f32, isOutput=True)

    with tile.TileContext(nc) as tc, ExitStack() as ctx:
        const = ctx.enter_context(tc.tile_pool(name="const", bufs=1))
        wraw = ctx.enter_context(tc.tile_pool(name="wraw", bufs=8))
        wm = ctx.enter_context(tc.tile_pool(name="wm", bufs=1))
        xts = ctx.enter_context(tc.tile_pool(name="xts", bufs=2))
        hb = ctx.enter_context(tc.tile_pool(name="hb", bufs=6))
        ob = ctx.enter_context(tc.tile_pool(name="ob", bufs=2))
        ph = ctx.enter_context(tc.tile_pool(name="ph", bufs=4, space="PSUM"))
        po = ctx.enter_context(tc.tile_pool(name="po", bufs=2, space="PSUM"))

        # ---- adjacency columns for this core: (logits > 0) * diag_mask ----
        adj = []
        for k in range(KC):
            lg_t = const.tile([128, VL], f32, tag=f"lg{k}")
            nc.sync.dma_start(out=lg_t, in_=LG[128 * k:128 * (k + 1), :])
            dm_t = const.tile([128, VL], f32, tag=f"dm{k}")
            nc.sync.dma_start(out=dm_t, in_=DM[128 * k:128 * (k + 1), :])
            a_t = const.tile([128, VL], f32, tag=f"adj{k}")
            nc.vector.tensor_scalar(
                out=a_t, in0=lg_t, scalar1=0.0, scalar2=None,
                op0=mybir.AluOpType.is_gt,
            )
            nc.vector.tensor_mul(a_t, a_t, dm_t)
            nc.sync.dma_start(out=adjO[128 * k:128 * (k + 1), :], in_=a_t)
            adj.append(a_t)

        # ---- biases ----
        # b1 arranged so that column p is the per-partition bias of pair p:
        # b1t[(par*64+h), p] = b1s[2p+par, h] = b1s.flat[128p + (par*64+h)]
        b1t = const.tile([128, PAIRS], f32, tag="b1t")
        nc.sync.dma_start(
            out=b1t, in_=b1s.rearrange("(p2 v2) h -> (v2 h) p2", v2=2)
        )
        # b2: local var vv -> PSUM partition vv
        b2t = const.tile([VL, 1], f32, tag="b2t")
        nc.sync.dma_start(out=b2t, in_=b2s[:].unsqueeze(1))

        # ---- layer-2 stationaries, one per pair ----
        # Pair p owns local vars 2p, 2p+1.  Its stationary [128, VL] is zero
        # except col 2p+par, which holds W2[2p+par] in partitions
        # [64*par, 64*par+64).  All 32 pairs accumulate into one [VL, 512]
        # PSUM bank (out partition = local variable index).
        w2p = []
        for p in range(PAIRS):
            stage = const.tile([128, VL], f32, tag=f"w2s{p}")
            nc.vector.memset(stage, 0.0)
            for par in range(2):
                nc.sync.dma_start(
                    out=stage[64 * par:64 * par + 64,
                              2 * p + par:2 * p + par + 1],
                    in_=W2s[2 * p + par, :].unsqueeze(1),
                )
            t = const.tile([128, VL], MM_DTYPE, tag=f"w2p{p}")
            nc.vector.tensor_copy(t, stage)
            w2p.append(t)

        # ---- masked W1 pair tiles, resident in SBUF ----
        # wmt[p][:, c, 64*par:64*par+64] = W1[2p+par, 128c+v, h] * adj[v, 2p+par]
        wmt = []
        for p in range(PAIRS):
            m = wm.tile([128, KC, 128], MM_DTYPE, tag=f"wm{p}")
            for par in range(2):
                i_loc = 2 * p + par
                raw = wraw.tile([128, KC, H], f32)
                nc.sync.dma_start(
                    out=raw, in_=W1s[i_loc].rearrange("(c p) h -> p c h", p=128)
                )
                for c in range(KC):
                    nc.vector.tensor_scalar_mul(
                        out=m[:, c, 64 * par:64 * par + 64],
                        in0=raw[:, c, :],
                        scalar1=adj[c][:, i_loc:i_loc + 1],
                    )
            wmt.append(m)

        # ---- main loop over batch tiles ----
        for t in [t for _ in range(repeat) for t in range(NBT)]:
            xt = []
            for c in range(KC):
                x_t = xts.tile([128, BTS], MM_DTYPE, tag=f"xt{c}")
                nc.gpsimd.dma_start(
                    out=x_t, in_=XT[128 * c:128 * (c + 1), BTS * t:BTS * (t + 1)]
                )
                xt.append(x_t)

            pout = po.tile([VL, BTS], f32)
            for p in range(PAIRS):
                psum_h = ph.tile([128, BTS], f32)
                for c in range(KC):
                    nc.tensor.matmul(
                        out=psum_h,
                        lhsT=wmt[p][:, c, :],
                        rhs=xt[c],
                        start=(c == 0),
                        stop=(c == KC - 1),
                    )
                h_t = hb.tile([128, BTS], MM_DTYPE)
                # h = max(psum + b1_pair, 0)  — one fused DVE op
                nc.vector.tensor_scalar(
                    out=h_t, in0=psum_h,
                    scalar1=b1t[:, p:p + 1], scalar2=0.0,
                    op0=mybir.AluOpType.add, op1=mybir.AluOpType.max,
                )
                nc.tensor.matmul(
                    out=pout,
                    lhsT=w2p[p],
                    rhs=h_t,
                    start=(p == 0),
                    stop=(p == PAIRS - 1),
                )

            o_t = ob.tile([VL, BTS], f32)
            nc.vector.tensor_scalar_add(out=o_t, in0=pout, scalar1=b2t[:, 0:1])
            nc.sync.dma_start(
                out=outT[:, BTS * t:BTS * (t + 1)], in_=o_t
            )

    nc.compile()
    return nc


_NC_CACHE = {}


def _get_program(repeat=1):
    if repeat not in _NC_CACHE:
        _NC_CACHE[repeat] = build_program(repeat)
    return _NC_CACHE[repeat]


def make_in_maps(X, adjacency_logits, W1, b1, W2, b2):
    X = np.asarray(X, np.float32)
    adjacency_logits = np.asarray(adjacency_logits, np.float32)
    W1 = np.asarray(W1, np.float32)
    b1 = np.asarray(b1, np.float32)
    W2 = np.asarray(W2, np.float32)
    b2 = np.asarray(b2, np.float32)

    XTh = np.ascontiguousarray(X.T)
    in_maps = []
    for c in range(NCORES):
        sl = slice(VL * c, VL * (c + 1))
        dm = np.ones((V, VL), np.float32)
        idx = np.arange(VL * c, VL * (c + 1))
        dm[idx, idx - VL * c] = 0.0
        in_maps.append({
            "XT": XTh,
            "W1s": np.ascontiguousarray(W1[sl]),
            "b1s": np.ascontiguousarray(b1[sl]),
            "W2s": np.ascontiguousarray(W2[sl]),
            "b2s": np.ascontiguousarray(b2[sl]),
            "LG": np.ascontiguousarray(adjacency_logits[:, sl]),
            "DM": dm,
        })
    return in_maps


def gather_results(results):
    out = np.concatenate([results[c]["outT"].T for c in range(NCORES)], axis=1)
    adj = np.concatenate([results[c]["adjO"] for c in range(NCORES)], axis=1)
    return out, adj


def kernel(X, adjacency_logits, W1, b1, W2, b2):
    from concourse.bass_utils import run_bass_kernel_spmd

    nc = _get_program()
    in_maps = make_in_maps(X, adjacency_logits, W1, b1, W2, b2)
    res = run_bass_kernel_spmd(nc, in_maps, list(range(NCORES)))
    return gather_results(res.results)


# revision 13
# speedup vs baseline: 420.3773x; 420.3773x over previous
"""Trainium2 Bass kernel for the DifferentiableDAG dense-MLP problem.

Computation (per variable i of V=512):
    adj = (sigmoid(logits) > 0.5) * (1 - eye)        # == (logits > 0) * (1-eye)
    W1_eff[i,v,h] = W1[i,v,h] * adj[v,i]
    h[b,i,:]   = relu(X @ W1_eff[i] + b1[i])         # [B,H]
    out[b,i]   = h[b,i,:] @ W2[i] + b2[i]            # [B]
returns (out [B,V], adj [V,V]).

Sharding: expert-parallel over the variable axis i — core c owns the 64
variables [64c, 64c+64).  X^T is replicated; outputs are gathered on host.

Per-core kernel layout ("transposed h"):
  layer 1: out_psum[(pair_var, h), b] = sum_v W1_eff^T X^T — stationary is the
  masked W1 pair tile [128v, 128=(2 vars x 64 h)], moving is X^T [128v, 512b],
  accumulated over 4 v-chunks.  Bias+relu is then ONE fused DVE tensor_scalar
  op (bias is per-partition in this layout).
  layer 2: per-pair stationary [128=(2x64h), 64] whose two non-zero columns
  hold the pair's W2 rows; all 32 pairs accumulate into one [64, 512] PSUM
  bank whose partition index is the local variable index.

All matmul inputs are float32r (TRN2 full-rate fp32 mode, ~1e-4 rel err):
produced either by casting gpsimd DMAs or by DVE ops writing f32r tiles.
"""

import numpy as np

import concourse.bass as bass
import concourse.mybir as mybir
import concourse.tile as tile
from contextlib import ExitStack

B, V, H = 4096, 512, 64
NCORES = 8
VL = V // NCORES          # 64 local variables per core
PAIRS = VL // 2           # 32
GROUPS = 4                # PSUM col-group strips for layer 2
PERG = PAIRS // GROUPS    # 8 pairs accumulated per strip
KC = V // 128             # 4 contraction chunks
BTS = 512                 # batch tile size
NBT = B // BTS            # 8 batch tiles

f32 = mybir.dt.float32

# Matmul input dtype: float32r streams at 1 cycle/row for N>=256 (4x faster
# than plain float32 on TRN2); numerics verified against the fp32 reference.
MM_DTYPE = mybir.dt.float32r




def build_program(repeat=1):
    """repeat>1 re-emits the main batch loop for slope-based HW timing."""
    from concourse import bacc
    nc = bacc.Bacc(
        "TRN2", target_bir_lowering=False, debug=False, num_devices=NCORES
    )

    XT = nc.declare_dram_parameter("XT", [V, B], f32, isOutput=False)
    W1s = nc.declare_dram_parameter("W1s", [VL, V, H], f32, isOutput=False)
    b1s = nc.declare_dram_parameter("b1s", [VL, H], f32, isOutput=False)
    W2s = nc.declare_dram_parameter("W2s", [VL, H], f32, isOutput=False)
    b2s = nc.declare_dram_parameter("b2s", [VL], f32, isOutput=False)
    LG = nc.declare_dram_parameter("LG", [V, VL], f32, isOutput=False)
    DM = nc.declare_dram_parameter("DM", [V, VL], f32, isOutput=False)
    outT = nc.declare_dram_parameter("outT", [VL, B], f32, isOutput=True)
    adjO = nc.declare_dram_parameter("adjO", [V, VL], f32, isOutput=True)

    with tile.TileContext(nc) as tc, ExitStack() as ctx:
        const = ctx.enter_context(tc.tile_pool(name="const", bufs=1))
        wraw = ctx.enter_context(tc.tile_pool(name="wraw", bufs=8))
        wm = ctx.enter_context(tc.tile_pool(name="wm", bufs=1))
        xts = ctx.enter_context(tc.tile_pool(name="xts", bufs=2))
        hb = ctx.enter_context(tc.tile_pool(name="hb", bufs=6))
        ob = ctx.enter_context(tc.tile_pool(name="ob", bufs=2))
        ph = ctx.enter_context(tc.tile_pool(name="ph", bufs=4, space="PSUM"))
        po = ctx.enter_context(tc.tile_pool(name="po", bufs=2, space="PSUM"))

        # ---- adjacency columns for this core: (logits > 0) * diag_mask ----
        adj = []
        for k in range(KC):
            lg_t = const.tile([128, VL], f32, tag=f"lg{k}")
            nc.sync.dma_start(out=lg_t, in_=LG[128 * k:128 * (k + 1), :])
            dm_t = const.tile([128, VL], f32, tag=f"dm{k}")
            nc.sync.dma_start(out=dm_t, in_=DM[128 * k:128 * (k + 1), :])
            a_t = const.tile([128, VL], f32, tag=f"adj{k}")
            nc.vector.tensor_scalar(
                out=a_t, in0=lg_t, scalar1=0.0, scalar2=None,
                op0=mybir.AluOpType.is_gt,
            )
            nc.vector.tensor_mul(a_t, a_t, dm_t)
            nc.sync.dma_start(out=adjO[128 * k:128 * (k + 1), :], in_=a_t)
            adj.append(a_t)

        # ---- biases ----
        # b1 arranged so that column p is the per-partition bias of pair p:
        # b1t[(par*64+h), p] = b1s[2p+par, h] = b1s.flat[128p + (par*64+h)]
        b1t = const.tile([128, PAIRS], f32, tag="b1t")
        nc.sync.dma_start(
            out=b1t, in_=b1s.rearrange("(p2 v2) h -> (v2 h) p2", v2=2)
        )
        # b2: local var vv -> PSUM partition vv
        b2t = const.tile([VL, 1], f32, tag="b2t")
        nc.sync.dma_start(out=b2t, in_=b2s[:].unsqueeze(1))

        # ---- layer-2 stationaries, one per pair ----
        # Pair p owns local vars 2p, 2p+1.  Its stationary [128, VL] is zero
        # except col 2p+par, which holds W2[2p+par] in partitions
        # [64*par, 64*par+64).  All 32 pairs accumulate into one [VL, 512]
        # PSUM bank (out partition = local variable index).
        w2p = []
        for p in range(PAIRS):
            stage = const.tile([128, VL], f32, tag=f"w2s{p}")
            nc.vector.memset(stage, 0.0)
            for par in range(2):
                nc.sync.dma_start(
                    out=stage[64 * par:64 * par + 64,
                              2 * p + par:2 * p + par + 1],
                    in_=W2s[2 * p + par, :].unsqueeze(1),
                )
            t = const.tile([128, VL], MM_DTYPE, tag=f"w2p{p}")
            nc.vector.tensor_copy(t, stage)
            w2p.append(t)

        # ---- masked W1 pair tiles, resident in SBUF ----
        # wmt[p][:, c, 64*par:64*par+64] = W1[2p+par, 128c+v, h] * adj[v, 2p+par]
        wmt = []
        for p in range(PAIRS):
            m = wm.tile([128, KC, 128], MM_DTYPE, tag=f"wm{p}")
            for par in range(2):
                i_loc = 2 * p + par
                raw = wraw.tile([128, KC, H], f32)
                nc.sync.dma_start(
                    out=raw, in_=W1s[i_loc].rearrange("(c p) h -> p c h", p=128)
                )
                for c in range(KC):
                    nc.vector.tensor_scalar_mul(
                        out=m[:, c, 64 * par:64 * par + 64],
                        in0=raw[:, c, :],
                        scalar1=adj[c][:, i_loc:i_loc + 1],
                    )
            wmt.append(m)

        # ---- main loop over batch tiles ----
        for t in [t for _ in range(repeat) for t in range(NBT)]:
            xt = []
            for c in range(KC):
                x_t = xts.tile([128, BTS], MM_DTYPE, tag=f"xt{c}")
                nc.gpsimd.dma_start(
                    out=x_t, in_=XT[128 * c:128 * (c + 1), BTS * t:BTS * (t + 1)]
                )
                xt.append(x_t)

            pout = po.tile([VL, BTS], f32)
            for p in range(PAIRS):
                psum_h = ph.tile([128, BTS], f32)
                for c in range(KC):
                    nc.tensor.matmul(
                        out=psum_h,
                        lhsT=wmt[p][:, c, :],
                        rhs=xt[c],
                        start=(c == 0),
                        stop=(c == KC - 1),
                    )
                h_t = hb.tile([128, BTS], MM_DTYPE)
                # h = max(psum + b1_pair, 0)  — one fused DVE op
                nc.vector.tensor_scalar(
                    out=h_t, in0=psum_h,
                    scalar1=b1t[:, p:p + 1], scalar2=0.0,
                    op0=mybir.AluOpType.add, op1=mybir.AluOpType.max,
                )
                nc.tensor.matmul(
                    out=pout,
                    lhsT=w2p[p],
                    rhs=h_t,
                    start=(p == 0),
                    stop=(p == PAIRS - 1),
                )

            o_t = ob.tile([VL, BTS], f32)
            nc.vector.tensor_scalar_add(out=o_t, in0=pout, scalar1=b2t[:, 0:1])
            nc.sync.dma_start(
                out=outT[:, BTS * t:BTS * (t + 1)], in_=o_t
            )

    nc.compile()
    return nc


_NC_CACHE = {}


def _get_program(repeat=1):
    if repeat not in _NC_CACHE:
        _NC_CACHE[repeat] = build_program(repeat)
    return _NC_CACHE[repeat]


def make_in_maps(X, adjacency_logits, W1, b1, W2, b2):
    X = np.asarray(X, np.float32)
    adjacency_logits = np.asarray(adjacency_logits, np.float32)
    W1 = np.asarray(W1, np.float32)
    b1 = np.asarray(b1, np.float32)
    W2 = np.asarray(W2, np.float32)
    b2 = np.asarray(b2, np.float32)

    XTh = np.ascontiguousarray(X.T)
    in_maps = []
    for c in range(NCORES):
        sl = slice(VL * c, VL * (c + 1))
        dm = np.ones((V, VL), np.float32)
        idx = np.arange(VL * c, VL * (c + 1))
        dm[idx, idx - VL * c] = 0.0
        in_maps.append({
            "XT": XTh,
            "W1s": np.ascontiguousarray(W1[sl]),
            "b1s": np.ascontiguousarray(b1[sl]),
            "W2s": np.ascontiguousarray(W2[sl]),
            "b2s": np.ascontiguousarray(b2[sl]),
            "LG": np.ascontiguousarray(adjacency_logits[:, sl]),
            "DM": dm,
        })
    return in_maps


def gather_results(results):
    out = np.concatenate([results[c]["outT"].T for c in range(NCORES)], axis=1)
    adj = np.concatenate([results[c]["adjO"] for c in range(NCORES)], axis=1)
    return out, adj


def kernel(X, adjacency_logits, W1, b1, W2, b2):
    from concourse.bass_utils import run_bass_kernel_spmd

    nc = _get_program()
    in_maps = make_in_maps(X, adjacency_logits, W1, b1, W2, b2)
    res = run_bass_kernel_spmd(nc, in_maps, list(range(NCORES)))
    return gather_results(res.results)
